# revision 10
# baseline (speedup 1.0000x reference)
"""MLA (multi-head latent attention) forward for trn2, 8-core SPMD.

Sharding: 128 heads -> 16 per core; batch (2) replicated (folded into the
token dim of the shared down-projections). All heavy matmuls in float32r
(1 cyc/row on PE at N>=512, ~2^-12 input truncation), fp32 PSUM accumulate.

Per core:
  phase 1a: qa^T = (Wqa x^T), ckv^T = (Wkva x^T)   [stationary weights]
            + sum-of-squares for the two RMSNorms via ones-matmuls
  phase 1b: r = 1/sqrt(mean+eps) (Sqrt ACT + DVE reciprocal); rank-1
            broadcast tiles R = ones^T r (norm scale applied post-matmul;
            g folded into up-proj weights host-side; softmax SCALE folded
            into Rq)
  phase 1c: q^T = Wqb qa^T (to DRAM scratch), k_nope^T = Wkvb_k ckv^T,
            V (natural layout) = ckv^T.T Wkvb_v, k_pe passthrough
  phase 2:  per (b, h): S^T[k,q] = K^T.T Q^T (+mask), E = exp(S^T),
            sumexp = ones^T E, O^T = V.T E, out = O^T * (1/sumexp)

Host side assembles full outputs (transposes, concat past K/V, k_pe
broadcast across heads).
"""
import sys

if "/opt/trn_rl_repo" not in sys.path:
    sys.path.insert(0, "/opt/trn_rl_repo")

import numpy as np

import concourse.bass as bass
import concourse.mybir as mybir
import concourse.tile as tile
from concourse import bacc
from concourse.bass_utils import run_bass_kernel_spmd

F32 = mybir.dt.float32
F32R = mybir.dt.float32r
EXP = mybir.ActivationFunctionType.Exp
SQRT = mybir.ActivationFunctionType.Sqrt

HID, QLORA, NH = 5120, 1536, 128
NOPE, ROPE, KVLORA = 128, 64, 512
QHD = NOPE + ROPE  # 192
VHD = 128
SCALE = QHD ** (-0.5) * 0.707
EPS = 1e-6
B, Q, PAST = 2, 512, 512
KTOT = PAST + Q  # 1024
T = B * Q  # 1024 tokens
NCORES = 8
HL = NH // NCORES  # 16 heads per core

KC_HID = HID // 128  # 40
MC_Q = QLORA // 128  # 12
MC_KV = 5  # ceil(576/128): 4 latent chunks + 1 (rope 64 + pad)
OC_Q = HL * QHD // 128  # 24
OC_K = HL * NOPE // 128  # 16
OV = HL * VHD  # 2048


def build_nc():
    nc = bacc.Bacc(None, target_bir_lowering=False, debug=False)

    # ---- DRAM I/O (f32r is bit-identical to f32 on the numpy side) ----
    xt_d = nc.dram_tensor("xt", [HID, T], F32R, kind="ExternalInput")
    wqa_d = nc.dram_tensor("wqa", [MC_Q, KC_HID, 128, 128], F32R, kind="ExternalInput")
    wkva_d = nc.dram_tensor("wkva", [MC_KV, KC_HID, 128, 128], F32R, kind="ExternalInput")
    wqb_d = nc.dram_tensor("wqb", [OC_Q, MC_Q, 128, 128], F32R, kind="ExternalInput")
    wkvbk_d = nc.dram_tensor("wkvbk", [OC_K, 4, 128, 128], F32R, kind="ExternalInput")
    wkvbv_d = nc.dram_tensor("wkvbv", [4, 128, OV], F32R, kind="ExternalInput")
    pkt_d = nc.dram_tensor("pkt", [B, HL, QHD, PAST], F32R, kind="ExternalInput")
    pv_d = nc.dram_tensor("pv", [B, HL, PAST, VHD], F32R, kind="ExternalInput")
    maskt_d = nc.dram_tensor("maskt", [B, KTOT, Q], F32, kind="ExternalInput")
    onesr_d = nc.dram_tensor("onesr", [128, 128], F32R, kind="ExternalInput")
    onesf_d = nc.dram_tensor("onesf", [1, 128], F32, kind="ExternalInput")

    qt_d = nc.dram_tensor("qt", [OC_Q * 128, T], F32R)  # scratch
    rkvt_d = nc.dram_tensor("rkvt", [8, 128], F32)  # scratch
    attn_d = nc.dram_tensor("attn", [B, HL, VHD, Q], F32, kind="ExternalOutput")
    knope_d = nc.dram_tensor("knope", [B, HL, NOPE, Q], F32, kind="ExternalOutput")
    kpe_d = nc.dram_tensor("kpe", [B, ROPE, Q], F32, kind="ExternalOutput")
    vnew_d = nc.dram_tensor("vnew", [B, HL, Q, VHD], F32, kind="ExternalOutput")

    with (
        tile.TileContext(nc) as tc,
        nc.allow_low_precision(reason="float32r tiles are bit-compatible fp32"),
    ):
        with tc.tile_pool(name="const", bufs=1) as constp:
            ones_r = constp.tile([128, 128], F32R, tag="onesr")
            nc.sync.dma_start(out=ones_r, in_=onesr_d[:])
            ones_f = constp.tile([1, 128], F32, tag="onesf")
            nc.sync.dma_start(out=ones_f, in_=onesf_d[:])
            biasq_t = constp.tile([1, 1], F32, tag="biasq")
            nc.vector.memset(biasq_t, EPS / (SCALE * SCALE))
            biaskv_t = constp.tile([1, 1], F32, tag="biaskv")
            nc.vector.memset(biaskv_t, EPS)

            acts_ctx = tc.tile_pool(name="acts", bufs=1)
            persist = acts_ctx.__enter__()
            qa_sb = persist.tile([128, MC_Q, T], F32R, tag="qa")  # 48K/part
            ckv_sb = persist.tile([128, MC_KV, T], F32R, tag="ckv")  # 20K/part
            rq_sb = persist.tile([1, T], F32, tag="rq")
            rkv_sb = persist.tile([1, T], F32, tag="rkv")
            rkvT_sb = persist.tile([128, 8], F32, tag="rkvT")
            Rq_sb = persist.tile([128, T], F32, tag="Rq")
            Rkv_sb = persist.tile([128, T], F32, tag="Rkv")

            # ---------------- phase 1a: down projections ----------------
            with (
                tc.tile_pool(name="ps1", bufs=1, space="PSUM") as ps1,
                tc.tile_pool(name="ps1b", bufs=3, space="PSUM") as ps1b,
            ):
                sumq_ps = ps1.tile([1, T], F32, tag="sumq")
                sumkv_ps = ps1.tile([1, T], F32, tag="sumkv")
                for th in range(2):
                    tsl = slice(th * 512, (th + 1) * 512)
                    with (
                        tc.tile_pool(name="xt", bufs=1) as xtp,
                        tc.tile_pool(name="w1", bufs=2) as w1p,
                        tc.tile_pool(name="sq", bufs=2) as sqp,
                    ):
                        xt_sb = xtp.tile([128, KC_HID, 512], F32R, tag="xt")
                        nc.sync.dma_start(
                            out=xt_sb,
                            in_=xt_d[:, tsl].rearrange("(k p) t -> p k t", p=128),
                        )
                        for m in range(MC_Q + MC_KV):
                            is_q = m < MC_Q
                            mk = m if is_q else m - MC_Q
                            pa = ps1b.tile([128, 512], F32, tag="pa")
                            for kh in range(2):  # weight tiles in k-halves
                                wt = w1p.tile([128, KC_HID // 2, 128], F32R, tag="w1")
                                src = (wqa_d if is_q else wkva_d)[
                                    mk, kh * 20 : (kh + 1) * 20
                                ]
                                nc.sync.dma_start(
                                    out=wt, in_=src.rearrange("k p c -> p k c")
                                )
                                for kb in range(20):
                                    nc.tensor.matmul(
                                        pa,
                                        wt[:, kb],
                                        xt_sb[:, kh * 20 + kb],
                                        start=(kh == 0 and kb == 0),
                                        stop=(kh == 1 and kb == 19),
                                    )
                            # stash activation (f32r; feeds MM2)
                            if is_q:
                                dest = qa_sb[:, mk, tsl]
                            else:
                                dest = ckv_sb[:, mk, tsl]
                            nc.vector.tensor_copy(out=dest, in_=pa)
                            if m == MC_Q + 4:
                                # rope rows of ckv: exact passthrough output
                                kpe_st = sqp.tile([ROPE, 512], F32, tag="kpe_st")
                                nc.scalar.copy(out=kpe_st, in_=pa[:ROPE])
                                nc.sync.dma_start(out=kpe_d[th], in_=kpe_st)
                                continue  # no rmsnorm stats for rope chunk
                            sq = sqp.tile([128, 512], F32R, tag="sq")
                            nc.scalar.activation(
                                sq, pa, mybir.ActivationFunctionType.Square
                            )
                            if is_q:
                                nc.tensor.matmul(
                                    sumq_ps[:, tsl],
                                    ones_r[:, :1],
                                    sq,
                                    start=(mk == 0),
                                    stop=(mk == MC_Q - 1),
                                )
                            else:
                                nc.tensor.matmul(
                                    sumkv_ps[:, tsl],
                                    ones_r[:, :1],
                                    sq,
                                    start=(mk == 0),
                                    stop=(mk == 3),
                                )
                # ---------------- phase 1b: norm factors ----------------
                # rq = SCALE / sqrt(mean(qa^2) + eps)
                #    = 1 / sqrt(sumq * (1/(QLORA*SCALE^2)) + eps/SCALE^2)
                sq_t = persist.tile([1, T], F32, tag="sq_t")
                nc.scalar.activation(
                    sq_t,
                    sumq_ps,
                    SQRT,
                    scale=1.0 / (QLORA * SCALE * SCALE),
                    bias=biasq_t,
                )
                nc.vector.reciprocal(rq_sb, sq_t)
                skv_t = persist.tile([1, T], F32, tag="skv_t")
                nc.scalar.activation(
                    skv_t, sumkv_ps, SQRT, scale=1.0 / KVLORA, bias=biaskv_t
                )
                nc.vector.reciprocal(rkv_sb, skv_t)
                # rkvT[p, j] = rkv[0, j*128+p]: reshape via DRAM bounce
                nc.sync.dma_start(
                    out=rkvt_d[:], in_=rkv_sb.rearrange("o (j p) -> o j p", p=128)
                )
                nc.sync.dma_start(
                    out=rkvT_sb, in_=rkvt_d.rearrange("j p -> p j")
                )

            # rank-1 broadcasts (exact fp32 matmuls)
            with tc.tile_pool(name="psR", bufs=2, space="PSUM") as psR:
                for th in range(2):
                    tsl = slice(th * 512, (th + 1) * 512)
                    Rq_ps = psR.tile([128, 512], F32, tag="R")
                    nc.tensor.matmul(Rq_ps, ones_f, rq_sb[:, tsl], start=True, stop=True)
                    nc.scalar.copy(out=Rq_sb[:, tsl], in_=Rq_ps)
                    Rkv_ps = psR.tile([128, 512], F32, tag="R")
                    nc.tensor.matmul(
                        Rkv_ps, ones_f, rkv_sb[:, tsl], start=True, stop=True
                    )
                    nc.scalar.copy(out=Rkv_sb[:, tsl], in_=Rkv_ps)

            # ---------------- phase 1c: up projections ----------------
            with (
                tc.tile_pool(name="w2", bufs=2) as w2p,
                tc.tile_pool(name="up_out", bufs=3) as outp,
                tc.tile_pool(name="ps2", bufs=4, space="PSUM") as ps2,
            ):
                # q^T -> DRAM scratch
                for o in range(OC_Q):
                    wt = w2p.tile([128, MC_Q, 128], F32R, tag="w2")
                    nc.sync.dma_start(
                        out=wt, in_=wqb_d[o].rearrange("k p c -> p k c")
                    )
                    for tcc in range(2):
                        tsl = slice(tcc * 512, (tcc + 1) * 512)
                        pq = ps2.tile([128, 512], F32, tag="pq")
                        for kb in range(MC_Q):
                            nc.tensor.matmul(
                                pq,
                                wt[:, kb],
                                qa_sb[:, kb, tsl],
                                start=(kb == 0),
                                stop=(kb == MC_Q - 1),
                            )
                        qsc = outp.tile([128, 512], F32R, tag="qsc")
                        nc.vector.tensor_mul(qsc, pq, Rq_sb[:, tsl])
                        nc.sync.dma_start(
                            out=qt_d[o * 128 : (o + 1) * 128, tsl], in_=qsc
                        )
                # k_nope^T -> output (tcc == batch)
                for o in range(OC_K):
                    wt = w2p.tile([128, 4, 128], F32R, tag="w2k")
                    nc.sync.dma_start(
                        out=wt, in_=wkvbk_d[o].rearrange("k p c -> p k c")
                    )
                    for tcc in range(2):
                        tsl = slice(tcc * 512, (tcc + 1) * 512)
                        pk = ps2.tile([128, 512], F32, tag="pq")
                        for kb in range(4):
                            nc.tensor.matmul(
                                pk,
                                wt[:, kb],
                                ckv_sb[:, kb, tsl],
                                start=(kb == 0),
                                stop=(kb == 3),
                            )
                        ksc = outp.tile([128, 512], F32, tag="ksc")
                        nc.vector.tensor_mul(ksc, pk, Rkv_sb[:, tsl])
                        nc.sync.dma_start(out=knope_d[tcc, o], in_=ksc)
                # V in natural [token, (h v)] layout -> output
                wvt = w2p.tile([128, 4, OV], F32R, tag="wv")  # 32K/part
                nc.sync.dma_start(out=wvt, in_=wkvbv_d.rearrange("c p o -> p c o"))
                for tch in range(8):
                    b = tch // 4
                    trow = (tch % 4) * 128
                    for oc in range(4):
                        pvt = ps2.tile([128, 512], F32, tag="pq")
                        for kb in range(4):
                            nc.tensor.matmul(
                                pvt,
                                ckv_sb[:, kb, tch * 128 : (tch + 1) * 128],
                                wvt[:, kb, oc * 512 : (oc + 1) * 512],
                                start=(kb == 0),
                                stop=(kb == 3),
                            )
                        vsc = outp.tile([128, 512], F32, tag="vsc")
                        nc.vector.tensor_scalar_mul(
                            vsc, pvt, rkvT_sb[:, tch : tch + 1]
                        )
                        nc.sync.dma_start(
                            out=vnew_d[
                                b, oc * 4 : (oc + 1) * 4, trow : trow + 128, :
                            ].rearrange("h t v -> t h v"),
                            in_=vsc.rearrange("t (h v) -> t h v", h=4),
                        )

            acts_ctx.__exit__(None, None, None)

            # ---------------- phase 2: attention ----------------
            with (
                tc.tile_pool(name="p2", bufs=3) as p2p,
                tc.tile_pool(name="p2e", bufs=2) as ep,
                tc.tile_pool(name="p2m", bufs=2) as mp,
                tc.tile_pool(name="psS", bufs=2, space="PSUM") as psS,
                tc.tile_pool(name="psO", bufs=2, space="PSUM") as psO,
            ):
                for b in range(B):
                    mask_sb = mp.tile([128, 8, 512], F32, tag="mask")
                    nc.sync.dma_start(
                        out=mask_sb,
                        in_=maskt_d[b].rearrange("(c p) q -> p c q", p=128),
                    )
                    for h in range(HL):
                        r0 = (h * QHD) // 128 * 0 + h * QHD  # row base in qt
                        qn = p2p.tile([128, 512], F32R, tag="qn")
                        nc.sync.dma_start(
                            out=qn, in_=qt_d[r0 : r0 + NOPE, b * 512 : (b + 1) * 512]
                        )
                        qr = p2p.tile([ROPE, 512], F32R, tag="qr")
                        nc.sync.dma_start(
                            out=qr,
                            in_=qt_d[r0 + NOPE : r0 + QHD, b * 512 : (b + 1) * 512],
                        )
                        kn = p2p.tile([128, KTOT], F32R, tag="kn")
                        nc.sync.dma_start(out=kn[:, :PAST], in_=pkt_d[b, h, :NOPE])
                        nc.gpsimd.dma_start(out=kn[:, PAST:], in_=knope_d[b, h])
                        kr = p2p.tile([ROPE, KTOT], F32R, tag="kr")
                        nc.sync.dma_start(out=kr[:, :PAST], in_=pkt_d[b, h, NOPE:])
                        nc.gpsimd.dma_start(out=kr[:, PAST:], in_=kpe_d[b])
                        vt_ = p2p.tile([128, 8, VHD], F32R, tag="v")
                        nc.sync.dma_start(
                            out=vt_[:, :4],
                            in_=pv_d[b, h].rearrange("(c p) v -> p c v", p=128),
                        )
                        nc.gpsimd.dma_start(
                            out=vt_[:, 4:],
                            in_=vnew_d[b, h].rearrange("(c p) v -> p c v", p=128),
                        )
                        E = ep.tile([128, 8, 512], F32R, tag="E")
                        for kc in range(8):
                            ksl = slice(kc * 128, (kc + 1) * 128)
                            sps = psS.tile([128, 512], F32, tag="s")
                            nc.tensor.matmul(sps, kn[:, ksl], qn, start=True, stop=False)
                            nc.tensor.matmul(sps, kr[:, ksl], qr, start=False, stop=True)
                            ein = p2p.tile([128, 512], F32, tag="ein")
                            nc.vector.tensor_add(ein, sps, mask_sb[:, kc])
                            nc.scalar.activation(E[:, kc], ein, EXP)
                        sum_ps = psO.tile([1, 512], F32, tag="sum")
                        o_ps = psO.tile([128, 512], F32, tag="o")
                        for kc in range(8):
                            nc.tensor.matmul(
                                sum_ps,
                                ones_r[:, :1],
                                E[:, kc],
                                start=(kc == 0),
                                stop=(kc == 7),
                            )
                            nc.tensor.matmul(
                                o_ps,
                                vt_[:, kc],
                                E[:, kc],
                                start=(kc == 0),
                                stop=(kc == 7),
                            )
                        rec = p2p.tile([1, 512], F32, tag="rec")
                        nc.vector.reciprocal(rec, sum_ps)
                        r_ps = psS.tile([128, 512], F32, tag="s")
                        nc.tensor.matmul(r_ps, ones_f, rec, start=True, stop=True)
                        rsb = p2p.tile([128, 512], F32, tag="rsb")
                        nc.scalar.copy(out=rsb, in_=r_ps)
                        att = p2p.tile([128, 512], F32, tag="att")
                        nc.vector.tensor_mul(att, o_ps, rsb)
                        nc.sync.dma_start(out=attn_d[b, h], in_=att)

    nc.compile()
    return nc


def host_prep(inputs):
    """Build the 8 per-core input maps from full inputs."""
    hidden = np.ascontiguousarray(inputs["hidden_states"], dtype=np.float32)
    mask = np.ascontiguousarray(inputs["attention_mask"], dtype=np.float32)
    past_key = np.ascontiguousarray(inputs["past_key"], dtype=np.float32)
    past_value = np.ascontiguousarray(inputs["past_value"], dtype=np.float32)
    w_q_a = np.ascontiguousarray(inputs["w_q_a"], dtype=np.float32)
    g_q_a = np.ascontiguousarray(inputs["g_q_a"], dtype=np.float32)
    w_q_b = np.ascontiguousarray(inputs["w_q_b"], dtype=np.float32)
    w_kv_a = np.ascontiguousarray(inputs["w_kv_a"], dtype=np.float32)
    g_kv_a = np.ascontiguousarray(inputs["g_kv_a"], dtype=np.float32)
    w_kv_b = np.ascontiguousarray(inputs["w_kv_b"], dtype=np.float32)

    xt = np.ascontiguousarray(hidden.reshape(T, HID).T)
    wqa = np.ascontiguousarray(
        w_q_a.reshape(MC_Q, 128, KC_HID, 128).transpose(0, 2, 3, 1)
    )
    wkva_p = np.zeros((MC_KV * 128, HID), np.float32)
    wkva_p[: KVLORA + ROPE] = w_kv_a
    wkva = np.ascontiguousarray(
        wkva_p.reshape(MC_KV, 128, KC_HID, 128).transpose(0, 2, 3, 1)
    )
    maskt = np.ascontiguousarray(mask[:, 0].transpose(0, 2, 1))  # [B, KTOT, Q]
    onesr = np.ones((128, 128), np.float32)
    onesf = np.ones((1, 128), np.float32)

    wqb_g = w_q_b * g_q_a[None, :]
    wkvb_g = (w_kv_b * g_kv_a[None, :]).reshape(NH, NOPE + VHD, KVLORA)

    in_maps = []
    for c in range(NCORES):
        h0 = c * HL
        wqb_c = wqb_g[h0 * QHD : (h0 + HL) * QHD]  # [3072, 1536]
        wqb_t = np.ascontiguousarray(
            wqb_c.reshape(OC_Q, 128, MC_Q, 128).transpose(0, 2, 3, 1)
        )
        kw = wkvb_g[h0 : h0 + HL, :NOPE]  # [16, 128, 512]
        wkvbk = np.ascontiguousarray(
            kw.reshape(OC_K, 128, 4, 128).transpose(0, 2, 3, 1)
        )
        vw = wkvb_g[h0 : h0 + HL, NOPE:]  # [16, 128, 512]
        wkvbv = np.ascontiguousarray(
            vw.transpose(2, 0, 1).reshape(4, 128, OV)
        )
        pkt = np.ascontiguousarray(
            past_key[:, h0 : h0 + HL].transpose(0, 1, 3, 2)
        )
        pv = np.ascontiguousarray(past_value[:, h0 : h0 + HL])
        in_maps.append(
            {
                "xt": xt,
                "wqa": wqa,
                "wkva": wkva,
                "wqb": wqb_t,
                "wkvbk": wkvbk,
                "wkvbv": wkvbv,
                "pkt": pkt,
                "pv": pv,
                "maskt": maskt,
                "onesr": onesr,
                "onesf": onesf,
            }
        )
    return in_maps


def assemble(inputs, results):
    """Gather per-core outputs into full-shape numpy outputs."""
    past_key = np.asarray(inputs["past_key"], dtype=np.float32)
    past_value = np.asarray(inputs["past_value"], dtype=np.float32)

    attn = np.empty((B, NH, Q, VHD), np.float32)
    key = np.empty((B, NH, KTOT, QHD), np.float32)
    val = np.empty((B, NH, KTOT, VHD), np.float32)
    key[:, :, :PAST] = past_key
    val[:, :, :PAST] = past_value
    for c in range(NCORES):
        h0 = c * HL
        r = results[c]
        attn[:, h0 : h0 + HL] = r["attn"].transpose(0, 1, 3, 2)
        key[:, h0 : h0 + HL, PAST:, :NOPE] = r["knope"].transpose(0, 1, 3, 2)
        key[:, h0 : h0 + HL, PAST:, NOPE:] = r["kpe"].transpose(0, 2, 1)[:, None]
        val[:, h0 : h0 + HL, PAST:] = r["vnew"]
    return attn, key, val


_NC_CACHE = {}


def kernel(**inputs):
    if "nc" not in _NC_CACHE:
        _NC_CACHE["nc"] = build_nc()
    nc = _NC_CACHE["nc"]
    in_maps = host_prep(inputs)
    res = run_bass_kernel_spmd(nc, in_maps, core_ids=list(range(NCORES)))
    return assemble(inputs, res.results)


# revision 14
# speedup vs baseline: 1.0734x; 1.0734x over previous
"""MLA (multi-head latent attention) forward for trn2, 8-core SPMD.

Sharding: 128 heads -> 16 per core; batch (2) replicated (folded into the
token dim). Down-projections sharded across cores over the output dim and
AllGathered (exact: f32 through the gather, bitwise f32->f32r on reload).
All heavy matmuls in float32r (1 cyc/row on PE at N=512, ~2^-12 input
truncation), fp32 PSUM accumulation.

Per core:
  phase 1a: qa^T slice = Wqa_c x^T, ckv^T slice = Wkva_c x^T  -> AllGather
  phase 1b: sum-of-squares via ones-matmuls; r = 1/sqrt(mean+eps)
            (Sqrt ACT + DVE reciprocal); rank-1 fp32 broadcast tiles
            (rmsnorm g folded into up-proj weights host-side; softmax
            SCALE folded into Rq)
  phase 1c: q^T = Wqb qa^T (SBUF-resident), k_nope^T = Wkvb_k ckv^T,
            V (natural layout) = ckv^T.T Wkvb_v, k_pe passthrough
  phase 2:  per (b, h): S^T[k,q] = K^T.T Q^T (+mask), E = exp(S^T),
            sumexp = ones^T E, O^T = V.T E, out = O^T * (1/sumexp)
            (no max-subtraction: scores are O(1) by construction)

Host assembles full outputs (transposes, concat past K/V, k_pe broadcast).
"""
import sys

if "/opt/trn_rl_repo" not in sys.path:
    sys.path.insert(0, "/opt/trn_rl_repo")

import numpy as np

import concourse.bass as bass
import concourse.mybir as mybir
import concourse.tile as tile
from concourse import bacc
from concourse.bass_utils import run_bass_kernel_spmd

F32 = mybir.dt.float32
F32R = mybir.dt.float32r
EXP = mybir.ActivationFunctionType.Exp
SQRT = mybir.ActivationFunctionType.Sqrt
SQUARE = mybir.ActivationFunctionType.Square

HID, QLORA, NH = 5120, 1536, 128
NOPE, ROPE, KVLORA = 128, 64, 512
QHD = NOPE + ROPE  # 192
VHD = 128
SCALE = QHD ** (-0.5) * 0.707
EPS = 1e-6
B, Q, PAST = 2, 512, 512
KTOT = PAST + Q  # 1024
T = B * Q  # 1024 tokens
NCORES = 8
HL = NH // NCORES  # 16 heads per core

KC_HID = HID // 128  # 40
MC_Q = QLORA // 128  # 12
MC_KV = 5  # ceil(576/128): 4 latent chunks + (rope 64 + pad)
OC_Q = HL * QHD // 128  # 24
OC_K = HL * NOPE // 128  # 16
OV = HL * VHD  # 2048
QSH = QLORA // NCORES  # 192 rows of qa per core
KVSH = 768 // NCORES  # 96 rows of (padded) ckv per core


def build_nc(use_mask=False):
    nc = bacc.Bacc(None, target_bir_lowering=False, debug=False)

    # ---- DRAM I/O (f32r is bit-identical to f32 on the numpy side) ----
    xt_d = nc.dram_tensor("xt", [HID, T], F32R, kind="ExternalInput")
    wqa_d = nc.dram_tensor("wqa", [2, KC_HID, 128, 96], F32R, kind="ExternalInput")
    wkva_d = nc.dram_tensor("wkva", [KC_HID, 128, 96], F32R, kind="ExternalInput")
    wqb_d = nc.dram_tensor("wqb", [OC_Q, MC_Q, 128, 128], F32R, kind="ExternalInput")
    wkvbk_d = nc.dram_tensor("wkvbk", [OC_K, 4, 128, 128], F32R, kind="ExternalInput")
    wkvbv_d = nc.dram_tensor("wkvbv", [4, 128, OV], F32R, kind="ExternalInput")
    pkt_d = nc.dram_tensor("pkt", [B, HL, QHD, PAST], F32R, kind="ExternalInput")
    pv_d = nc.dram_tensor("pv", [B, HL, PAST, VHD], F32R, kind="ExternalInput")
    onesr_d = nc.dram_tensor("onesr", [128, 128], F32R, kind="ExternalInput")
    onesf_d = nc.dram_tensor("onesf", [1, 128], F32, kind="ExternalInput")
    if use_mask:
        maskt_d = nc.dram_tensor("maskt", [B, KTOT, Q], F32, kind="ExternalInput")

    qaloc_d = nc.dram_tensor("qaloc", [2, QSH, 512], F32)
    ckvloc_d = nc.dram_tensor("ckvloc", [2, KVSH, 512], F32)
    qaall_d = nc.dram_tensor("qaall", [2, QLORA, 512], F32, addr_space="Shared")
    ckvall_d = nc.dram_tensor("ckvall", [2, 768, 512], F32, addr_space="Shared")
    rkvt_d = nc.dram_tensor("rkvt", [8, 128], F32)  # scratch
    qt_d = nc.dram_tensor("qt", [OC_Q * 128, T], F32R)  # q^T scratch

    attn_d = nc.dram_tensor("attn", [B, HL, VHD, Q], F32, kind="ExternalOutput")
    knope_d = nc.dram_tensor("knope", [B, HL, NOPE, Q], F32, kind="ExternalOutput")
    kpe_d = nc.dram_tensor("kpe", [B, ROPE, Q], F32, kind="ExternalOutput")
    vnew_d = nc.dram_tensor("vnew", [B, HL, Q, VHD], F32, kind="ExternalOutput")

    with (
        tile.TileContext(nc) as tc,
        nc.allow_low_precision(reason="float32r tiles are bit-compatible fp32"),
    ):
        with tc.tile_pool(name="const", bufs=1) as constp:
            ones_r = constp.tile([128, 128], F32R, tag="onesr")
            nc.sync.dma_start(out=ones_r, in_=onesr_d[:])
            ones_f = constp.tile([1, 128], F32, tag="onesf")
            nc.sync.dma_start(out=ones_f, in_=onesf_d[:])
            biasq_t = constp.tile([1, 1], F32, tag="biasq")
            nc.vector.memset(biasq_t, EPS / (SCALE * SCALE))
            biaskv_t = constp.tile([1, 1], F32, tag="biaskv")
            nc.vector.memset(biaskv_t, EPS)

            acts_ctx = tc.tile_pool(name="acts", bufs=1)
            persist = acts_ctx.__enter__()
            qa_sb = persist.tile([128, MC_Q, T], F32R, tag="qa")  # 48K/part
            ckv_sb = persist.tile([128, MC_KV, T], F32R, tag="ckv")  # 20K/part
            rq_sb = persist.tile([1, T], F32, tag="rq")
            rkv_sb = persist.tile([1, T], F32, tag="rkv")
            rkvT_sb = persist.tile([128, 8], F32, tag="rkvT")
            Rq_sb = persist.tile([128, T], F32, tag="Rq")
            Rkv_sb = persist.tile([128, T], F32, tag="Rkv")

            # ------- phase 1a: sharded down projections + AllGather -------
            with tc.tile_pool(name="ps1b", bufs=3, space="PSUM") as ps1b:
                for th in range(2):
                    tsl = slice(th * 512, (th + 1) * 512)
                    with (
                        tc.tile_pool(name="xt", bufs=1) as xtp,
                        tc.tile_pool(name="w1", bufs=2) as w1p,
                        tc.tile_pool(name="st1", bufs=2) as st1p,
                    ):
                        xt_sb = xtp.tile([128, KC_HID, 512], F32R, tag="xt")
                        nc.sync.dma_start(
                            out=xt_sb,
                            in_=xt_d[:, tsl].rearrange("(k p) t -> p k t", p=128),
                        )
                        for m in range(3):  # 2 qa tiles + 1 ckv tile, 96 rows each
                            pa = ps1b.tile([96, 512], F32, tag="pa")
                            for kh in range(2):
                                wt = w1p.tile([128, KC_HID // 2, 96], F32R, tag="w1")
                                src = (
                                    wqa_d[m, kh * 20 : (kh + 1) * 20]
                                    if m < 2
                                    else wkva_d[kh * 20 : (kh + 1) * 20]
                                )
                                nc.sync.dma_start(
                                    out=wt, in_=src.rearrange("k p c -> p k c")
                                )
                                for kb in range(20):
                                    nc.tensor.matmul(
                                        pa,
                                        wt[:, kb],
                                        xt_sb[:, kh * 20 + kb],
                                        start=(kh == 0 and kb == 0),
                                        stop=(kh == 1 and kb == 19),
                                    )
                            stg = st1p.tile([96, 512], F32, tag="stg")
                            nc.vector.tensor_copy(out=stg, in_=pa)
                            if m < 2:
                                nc.sync.dma_start(
                                    out=qaloc_d[th, m * 96 : (m + 1) * 96], in_=stg
                                )
                            else:
                                nc.sync.dma_start(out=ckvloc_d[th], in_=stg)
                    nc.gpsimd.collective_compute(
                        "AllGather",
                        mybir.AluOpType.bypass,
                        replica_groups=[list(range(NCORES))],
                        ins=[qaloc_d[th]],
                        outs=[qaall_d[th]],
                    )
                    nc.gpsimd.collective_compute(
                        "AllGather",
                        mybir.AluOpType.bypass,
                        replica_groups=[list(range(NCORES))],
                        ins=[ckvloc_d[th]],
                        outs=[ckvall_d[th]],
                    )
                # reload gathered activations (bitwise f32 -> f32r cast)
                for th in range(2):
                    tsl = slice(th * 512, (th + 1) * 512)
                    for m in range(MC_Q):
                        nc.gpsimd.dma_start(
                            out=qa_sb[:, m, tsl],
                            in_=qaall_d[th, m * 128 : (m + 1) * 128],
                        )
                    for m in range(MC_KV):
                        nc.gpsimd.dma_start(
                            out=ckv_sb[:, m, tsl],
                            in_=ckvall_d[th, m * 128 : (m + 1) * 128],
                        )

                # ------- phase 1b: rmsnorm stats + factors -------
                with (
                    tc.tile_pool(name="ps1s", bufs=1, space="PSUM") as ps1s,
                    tc.tile_pool(name="sq", bufs=3) as sqp,
                    tc.tile_pool(name="kpest", bufs=2) as kpestp,
                ):
                    sumq_ps = ps1s.tile([1, T], F32, tag="sumq")
                    sumkv_ps = ps1s.tile([1, T], F32, tag="sumkv")
                    for th in range(2):
                        tsl = slice(th * 512, (th + 1) * 512)
                        # k_pe exact passthrough (from the f32 gather buffer)
                        kpe_st = kpestp.tile([ROPE, 512], F32, tag="kpest")
                        nc.sync.dma_start(
                            out=kpe_st, in_=ckvall_d[th, KVLORA : KVLORA + ROPE]
                        )
                        nc.sync.dma_start(out=kpe_d[th], in_=kpe_st)
                        for m in range(MC_Q):
                            sq = sqp.tile([128, 512], F32R, tag="sq")
                            nc.scalar.activation(sq, qa_sb[:, m, tsl], SQUARE)
                            nc.tensor.matmul(
                                sumq_ps[:, tsl],
                                ones_r[:, :1],
                                sq,
                                start=(m == 0),
                                stop=(m == MC_Q - 1),
                            )
                        for m in range(4):
                            sq = sqp.tile([128, 512], F32R, tag="sq")
                            nc.scalar.activation(sq, ckv_sb[:, m, tsl], SQUARE)
                            nc.tensor.matmul(
                                sumkv_ps[:, tsl],
                                ones_r[:, :1],
                                sq,
                                start=(m == 0),
                                stop=(m == 3),
                            )
                    # rq = SCALE/sqrt(mean+eps) = 1/sqrt(sumq/(QLORA*SCALE^2)
                    #      + eps/SCALE^2); Rsqrt ACT is banned -> Sqrt+recip
                    sq_t = persist.tile([1, T], F32, tag="sq_t")
                    nc.scalar.activation(
                        sq_t, sumq_ps, SQRT,
                        scale=1.0 / (QLORA * SCALE * SCALE), bias=biasq_t,
                    )
                    nc.vector.reciprocal(rq_sb, sq_t)
                    skv_t = persist.tile([1, T], F32, tag="skv_t")
                    nc.scalar.activation(
                        skv_t, sumkv_ps, SQRT, scale=1.0 / KVLORA, bias=biaskv_t
                    )
                    nc.vector.reciprocal(rkv_sb, skv_t)
                    # rkvT[p, j] = rkv[0, j*128+p]: reshape via DRAM bounce
                    nc.sync.dma_start(
                        out=rkvt_d[:],
                        in_=rkv_sb.rearrange("o (j p) -> o j p", p=128),
                    )
                    nc.sync.dma_start(
                        out=rkvT_sb, in_=rkvt_d.rearrange("j p -> p j")
                    )

            # rank-1 broadcasts (exact fp32 matmuls)
            with tc.tile_pool(name="psR", bufs=2, space="PSUM") as psR:
                for th in range(2):
                    tsl = slice(th * 512, (th + 1) * 512)
                    Rq_ps = psR.tile([128, 512], F32, tag="R")
                    nc.tensor.matmul(Rq_ps, ones_f, rq_sb[:, tsl], start=True, stop=True)
                    nc.scalar.copy(out=Rq_sb[:, tsl], in_=Rq_ps)
                    Rkv_ps = psR.tile([128, 512], F32, tag="R")
                    nc.tensor.matmul(
                        Rkv_ps, ones_f, rkv_sb[:, tsl], start=True, stop=True
                    )
                    nc.scalar.copy(out=Rkv_sb[:, tsl], in_=Rkv_ps)

            # ---------------- phase 1c: up projections ----------------
            with (
                tc.tile_pool(name="w2", bufs=2) as w2p,
                tc.tile_pool(name="up_out", bufs=3) as outp,
                tc.tile_pool(name="ps2", bufs=4, space="PSUM") as ps2,
            ):
                # q^T -> SBUF-resident q_res (scaled by Rq = SCALE*rq)
                for o in range(OC_Q):
                    wt = w2p.tile([128, MC_Q, 128], F32R, tag="w2")
                    nc.sync.dma_start(
                        out=wt, in_=wqb_d[o].rearrange("k p c -> p k c")
                    )
                    for tcc in range(2):
                        tsl = slice(tcc * 512, (tcc + 1) * 512)
                        pq = ps2.tile([128, 512], F32, tag="pq")
                        for kb in range(MC_Q):
                            nc.tensor.matmul(
                                pq,
                                wt[:, kb],
                                qa_sb[:, kb, tsl],
                                start=(kb == 0),
                                stop=(kb == MC_Q - 1),
                            )
                        qsc = outp.tile([128, 512], F32R, tag="qsc")
                        nc.vector.tensor_mul(qsc, pq, Rq_sb[:, tsl])
                        nc.sync.dma_start(
                            out=qt_d[o * 128 : (o + 1) * 128, tsl], in_=qsc
                        )
                # k_nope^T -> output (tcc == batch)
                for o in range(OC_K):
                    wt = w2p.tile([128, 4, 128], F32R, tag="w2k")
                    nc.sync.dma_start(
                        out=wt, in_=wkvbk_d[o].rearrange("k p c -> p k c")
                    )
                    for tcc in range(2):
                        tsl = slice(tcc * 512, (tcc + 1) * 512)
                        pk = ps2.tile([128, 512], F32, tag="pq")
                        for kb in range(4):
                            nc.tensor.matmul(
                                pk,
                                wt[:, kb],
                                ckv_sb[:, kb, tsl],
                                start=(kb == 0),
                                stop=(kb == 3),
                            )
                        ksc = outp.tile([128, 512], F32, tag="ksc")
                        nc.vector.tensor_mul(ksc, pk, Rkv_sb[:, tsl])
                        nc.sync.dma_start(out=knope_d[tcc, o], in_=ksc)
                # V in natural [token, (h v)] layout -> output
                wvt = w2p.tile([128, 4, OV], F32R, tag="wv")  # 32K/part
                nc.sync.dma_start(out=wvt, in_=wkvbv_d.rearrange("c p o -> p c o"))
                for tch in range(8):
                    b = tch // 4
                    trow = (tch % 4) * 128
                    for oc in range(4):
                        pvt = ps2.tile([128, 512], F32, tag="pq")
                        for kb in range(4):
                            nc.tensor.matmul(
                                pvt,
                                ckv_sb[:, kb, tch * 128 : (tch + 1) * 128],
                                wvt[:, kb, oc * 512 : (oc + 1) * 512],
                                start=(kb == 0),
                                stop=(kb == 3),
                            )
                        vsc = outp.tile([128, 512], F32, tag="vsc")
                        nc.vector.tensor_scalar_mul(
                            vsc, pvt, rkvT_sb[:, tch : tch + 1]
                        )
                        nc.sync.dma_start(
                            out=vnew_d[
                                b, oc * 4 : (oc + 1) * 4, trow : trow + 128, :
                            ].rearrange("h t v -> t h v"),
                            in_=vsc.rearrange("t (h v) -> t h v", h=4),
                        )

            acts_ctx.__exit__(None, None, None)

            # ---------------- phase 2: attention ----------------
            p2_bufs = 2 if use_mask else 3
            with (
                tc.tile_pool(name="p2", bufs=p2_bufs) as p2p,
                tc.tile_pool(name="p2e", bufs=2) as ep,
                tc.tile_pool(name="psS", bufs=2, space="PSUM") as psS,
                tc.tile_pool(name="psO", bufs=2, space="PSUM") as psO,
            ):
                mp_ctx = tc.tile_pool(name="p2m", bufs=2) if use_mask else None
                mp = mp_ctx.__enter__() if mp_ctx else None
                for b in range(B):
                    bsl = slice(b * 512, (b + 1) * 512)
                    if use_mask:
                        mask_sb = mp.tile([128, 8, 512], F32, tag="mask")
                        nc.sync.dma_start(
                            out=mask_sb,
                            in_=maskt_d[b].rearrange("(c p) q -> p c q", p=128),
                        )
                    for h in range(HL):
                        r0q = h * QHD
                        qn = p2p.tile([128, 512], F32R, tag="qn")
                        nc.sync.dma_start(
                            out=qn, in_=qt_d[r0q : r0q + NOPE, bsl]
                        )
                        qr = p2p.tile([ROPE, 512], F32R, tag="qr")
                        nc.sync.dma_start(
                            out=qr, in_=qt_d[r0q + NOPE : r0q + QHD, bsl]
                        )
                        qsegs = [(qn, 0, 128, 0), (qr, 128, ROPE, 0)]
                        ktiles = []
                        for si, (qseg, d0, dn, p0) in enumerate(qsegs):
                            # K rows live at the same partition offset as the
                            # matching q segment
                            kt = p2p.tile([128, KTOT], F32R, tag=f"k{si}")
                            nc.sync.dma_start(
                                out=kt[p0 : p0 + dn, :PAST],
                                in_=pkt_d[b, h, d0 : d0 + dn],
                            )
                            # new rows: nope from knope_d, rope from kpe_d
                            n_nope = min(dn, max(0, NOPE - d0))
                            if n_nope > 0:
                                nc.gpsimd.dma_start(
                                    out=kt[p0 : p0 + n_nope, PAST:],
                                    in_=knope_d[b, h, d0 : d0 + n_nope],
                                )
                            if d0 + dn > NOPE:
                                r0 = max(0, NOPE - d0)
                                nc.gpsimd.dma_start(
                                    out=kt[p0 + r0 : p0 + dn, PAST:],
                                    in_=kpe_d[b, max(0, d0 - NOPE) : d0 + dn - NOPE],
                                )
                            ktiles.append((kt[p0 : p0 + dn], qseg))
                        vt_ = p2p.tile([128, 8, VHD], F32R, tag="v")
                        nc.sync.dma_start(
                            out=vt_[:, :4],
                            in_=pv_d[b, h].rearrange("(c p) v -> p c v", p=128),
                        )
                        nc.gpsimd.dma_start(
                            out=vt_[:, 4:],
                            in_=vnew_d[b, h].rearrange("(c p) v -> p c v", p=128),
                        )
                        E = ep.tile([128, 8, 512], F32R, tag="E")
                        for kc in range(8):
                            ksl = slice(kc * 128, (kc + 1) * 128)
                            sps = psS.tile([128, 512], F32, tag="s")
                            for i, (ktseg, qseg) in enumerate(ktiles):
                                nc.tensor.matmul(
                                    sps,
                                    ktseg[:, ksl],
                                    qseg,
                                    start=(i == 0),
                                    stop=(i == len(ktiles) - 1),
                                )
                            if use_mask:
                                ein = p2p.tile([128, 512], F32, tag="ein")
                                nc.vector.tensor_add(ein, sps, mask_sb[:, kc])
                                nc.scalar.activation(E[:, kc], ein, EXP)
                            else:
                                nc.scalar.activation(E[:, kc], sps, EXP)
                        sum_ps = psO.tile([1, 512], F32, tag="sum")
                        o_ps = psO.tile([128, 512], F32, tag="o")
                        for kc in range(8):
                            nc.tensor.matmul(
                                sum_ps,
                                ones_r[:, :1],
                                E[:, kc],
                                start=(kc == 0),
                                stop=(kc == 7),
                            )
                            nc.tensor.matmul(
                                o_ps,
                                vt_[:, kc],
                                E[:, kc],
                                start=(kc == 0),
                                stop=(kc == 7),
                            )
                        rec = p2p.tile([1, 512], F32, tag="rec")
                        nc.vector.reciprocal(rec, sum_ps)
                        r_ps = psS.tile([128, 512], F32, tag="s")
                        nc.tensor.matmul(r_ps, ones_f, rec, start=True, stop=True)
                        rsb = p2p.tile([128, 512], F32, tag="rsb")
                        nc.scalar.copy(out=rsb, in_=r_ps)
                        att = p2p.tile([128, 512], F32, tag="att")
                        nc.vector.tensor_mul(att, o_ps, rsb)
                        nc.sync.dma_start(out=attn_d[b, h], in_=att)
                if mp_ctx:
                    mp_ctx.__exit__(None, None, None)

    nc.compile()
    return nc


def host_prep(inputs, use_mask):
    """Build the 8 per-core input maps from full inputs."""
    hidden = np.ascontiguousarray(inputs["hidden_states"], dtype=np.float32)
    past_key = np.ascontiguousarray(inputs["past_key"], dtype=np.float32)
    past_value = np.ascontiguousarray(inputs["past_value"], dtype=np.float32)
    w_q_a = np.ascontiguousarray(inputs["w_q_a"], dtype=np.float32)
    g_q_a = np.ascontiguousarray(inputs["g_q_a"], dtype=np.float32)
    w_q_b = np.ascontiguousarray(inputs["w_q_b"], dtype=np.float32)
    w_kv_a = np.ascontiguousarray(inputs["w_kv_a"], dtype=np.float32)
    g_kv_a = np.ascontiguousarray(inputs["g_kv_a"], dtype=np.float32)
    w_kv_b = np.ascontiguousarray(inputs["w_kv_b"], dtype=np.float32)

    xt = np.ascontiguousarray(hidden.reshape(T, HID).T)
    wkva_pad = np.zeros((768, HID), np.float32)
    wkva_pad[: KVLORA + ROPE] = w_kv_a
    onesr = np.ones((128, 128), np.float32)
    onesf = np.ones((1, 128), np.float32)
    if use_mask:
        mask = np.ascontiguousarray(inputs["attention_mask"], dtype=np.float32)
        maskt = np.ascontiguousarray(mask[:, 0].transpose(0, 2, 1))

    wqb_g = w_q_b * g_q_a[None, :]
    wkvb_g = (w_kv_b * g_kv_a[None, :]).reshape(NH, NOPE + VHD, KVLORA)

    in_maps = []
    for c in range(NCORES):
        h0 = c * HL
        wqa_c = np.ascontiguousarray(
            w_q_a[c * QSH : (c + 1) * QSH]
            .reshape(2, 96, KC_HID, 128)
            .transpose(0, 2, 3, 1)
        )
        wkva_c = np.ascontiguousarray(
            wkva_pad[c * KVSH : (c + 1) * KVSH]
            .reshape(96, KC_HID, 128)
            .transpose(1, 2, 0)
        )
        wqb_c = wqb_g[h0 * QHD : (h0 + HL) * QHD]  # [3072, 1536]
        wqb_t = np.ascontiguousarray(
            wqb_c.reshape(OC_Q, 128, MC_Q, 128).transpose(0, 2, 3, 1)
        )
        kw = wkvb_g[h0 : h0 + HL, :NOPE]  # [16, 128, 512]
        wkvbk = np.ascontiguousarray(
            kw.reshape(OC_K, 128, 4, 128).transpose(0, 2, 3, 1)
        )
        vw = wkvb_g[h0 : h0 + HL, NOPE:]  # [16, 128, 512]
        wkvbv = np.ascontiguousarray(vw.transpose(2, 0, 1).reshape(4, 128, OV))
        pkt = np.ascontiguousarray(past_key[:, h0 : h0 + HL].transpose(0, 1, 3, 2))
        pv = np.ascontiguousarray(past_value[:, h0 : h0 + HL])
        im = {
            "xt": xt,
            "wqa": wqa_c,
            "wkva": wkva_c,
            "wqb": wqb_t,
            "wkvbk": wkvbk,
            "wkvbv": wkvbv,
            "pkt": pkt,
            "pv": pv,
            "onesr": onesr,
            "onesf": onesf,
        }
        if use_mask:
            im["maskt"] = maskt
        in_maps.append(im)
    return in_maps


def assemble(inputs, results):
    """Gather per-core outputs into full-shape numpy outputs."""
    past_key = np.asarray(inputs["past_key"], dtype=np.float32)
    past_value = np.asarray(inputs["past_value"], dtype=np.float32)

    attn = np.empty((B, NH, Q, VHD), np.float32)
    key = np.empty((B, NH, KTOT, QHD), np.float32)
    val = np.empty((B, NH, KTOT, VHD), np.float32)
    key[:, :, :PAST] = past_key
    val[:, :, :PAST] = past_value
    for c in range(NCORES):
        h0 = c * HL
        r = results[c]
        attn[:, h0 : h0 + HL] = r["attn"].transpose(0, 1, 3, 2)
        key[:, h0 : h0 + HL, PAST:, :NOPE] = r["knope"].transpose(0, 1, 3, 2)
        key[:, h0 : h0 + HL, PAST:, NOPE:] = r["kpe"].transpose(0, 2, 1)[:, None]
        val[:, h0 : h0 + HL, PAST:] = r["vnew"]
    return attn, key, val


_NC_CACHE = {}


def kernel(**inputs):
    use_mask = bool(np.any(np.asarray(inputs["attention_mask"])))
    if use_mask not in _NC_CACHE:
        _NC_CACHE[use_mask] = build_nc(use_mask)
    nc = _NC_CACHE[use_mask]
    in_maps = host_prep(inputs, use_mask)
    res = run_bass_kernel_spmd(nc, in_maps, core_ids=list(range(NCORES)))
    return assemble(inputs, res.results)
